# revision 15
# baseline (speedup 1.0000x reference)
"""DILATE loss (soft-DTW value + path) Trainium2 Bass kernel, v2.

1024 independent (b, f) soft-DTW problems, 128 per core, one per SBUF
partition. Row-major layout (row stride S=129 incl. left/top borders).

Key ideas vs v1:
- The sequential backward E-recursion is GONE. Path weights come from the
  posterior identity  E = exp((R_NN + D - R - Rbar)/gamma)  where Rbar is a
  REVERSE soft-DTW DP (forward DP on the flipped cost matrix). Fwd and rev
  DPs are independent -> interleaved per wavefront step, so the two chains
  pipeline across engines.
- D is built once with two whole-matrix ops (broadcast sub on DVE + Square
  on ACT); DP steps read its anti-diagonals via strided APs. The reverse
  chain reads the same D with negated strides.
- Rbar is written mirror-flat (negative-stride writes), so the E-phase is
  four whole-matrix chunk ops: two subs, one exp (bias = R_NN*100 per
  partition), and one fused multiply-by-(i-j)^2+accumulate against a
  shifted view of a (2N+1) squares table.
- The activation table holding BOTH exp and ln is preloaded once
  (act_func_set_id=6); without it the compiler alternates exp-only /
  ln-only tables at ~1.3us per reload.
"""
import sys

for _p in ("/opt/trn_rl_repo", "/root/.axon_site/_ro/trn_rl_repo"):
    if _p not in sys.path:
        sys.path.append(_p)

import numpy as np

N = 128
S = N + 1          # row stride of R / R2 (col 0 = left border)
RSZ = (N + 1) * S  # 16641; row 0 = top border; R[N,N] at flat RSZ-1
DSZ = N * N
NCORES = 8
GAMMA = 0.01
BIG = 1e8
INV_G = 100.0


def build_kernel(tc, out_ap, t_ap, o_ap):
    import concourse.bass as bass
    import concourse.mybir as mybir
    from concourse.ap import AP

    nc = tc.nc
    dt = mybir.dt.float32
    AF = mybir.ActivationFunctionType
    ALU = mybir.AluOpType

    # Preload the activation table containing BOTH exp and ln.
    nc.scalar.add_instruction(mybir.InstLoadActFuncSet(
        name=nc.scalar.bass.get_next_instruction_name(),
        act_func_set_id=6, ins=[], outs=[]))

    def sl(tile, base, pairs):
        a = tile[:]
        return AP(a.tensor, a.offset + base, [list(a.ap[0])] + list(pairs))

    from contextlib import ExitStack
    ctx = ExitStack()
    with ctx:
        persist = ctx.enter_context(tc.tile_pool(name="persist", bufs=1))

        Rt = persist.tile([128, RSZ], dt, tag="Rt")
        R2 = persist.tile([128, RSZ], dt, tag="R2")
        Dt = persist.tile([128, DSZ], dt, tag="Dt")
        tT = persist.tile([128, N], dt, tag="tT")
        oT = persist.tile([128, N], dt, tag="oT")
        SQf = persist.tile([128, 2 * N + 1], dt, tag="SQf")
        rnnI = persist.tile([128, 1], dt, tag="rnnI")
        acc = persist.tile([128, 4], dt, tag="acc")
        outt = persist.tile([128, 2], dt, tag="outt")

        # ping-pong work tiles per chain: A3 (0:384) + ss (384:512) fused,
        # m1 (reused as mdc), m
        NB = 2
        A3w = [[None] * NB for _ in range(2)]
        m1w = [[None] * NB for _ in range(2)]
        mw = [[None] * NB for _ in range(2)]
        for c in range(2):
            for b in range(NB):
                A3w[c][b] = persist.tile([128, 512], dt, tag=f"A3_{c}_{b}",
                                         name=f"A3_{c}_{b}")
            m1w[c][0] = m1w[c][1] = persist.tile(
                [128, N], dt, tag=f"m1_{c}", name=f"m1_{c}")
            mw[c][0] = mw[c][1] = persist.tile(
                [128, N], dt, tag=f"m_{c}", name=f"m_{c}")

        # ---- setup ----
        nc.sync.dma_start(tT[:], t_ap[:])
        nc.sync.dma_start(oT[:], o_ap[:])
        nc.vector.memset(Rt[:], BIG)
        nc.vector.memset(Rt[:, 0:1], 0.0)
        nc.vector.memset(R2[:, 0:RSZ], BIG)
        nc.vector.memset(R2[:, RSZ - 1:RSZ], 0.0)
        nc.gpsimd.iota(SQf[:].bitcast(mybir.dt.int32),
                       pattern=[[1, 2 * N + 1]], base=0,
                       channel_multiplier=0)
        nc.gpsimd.tensor_copy(SQf[:], SQf[:].bitcast(mybir.dt.int32))
        nc.gpsimd.tensor_scalar(out=SQf[:], in0=SQf[:], scalar1=float(N),
                                scalar2=0.0, op0=ALU.subtract, op1=ALU.add)
        nc.gpsimd.tensor_mul(SQf[:], SQf[:], SQf[:])

        # ---- D = (t_i - o_j)^2, whole matrix ----
        tb = AP(tT[:].tensor, tT[:].offset,
                [list(tT[:].ap[0]), [1, N], [0, N]])
        ob = AP(oT[:].tensor, oT[:].offset,
                [list(oT[:].ap[0]), [0, N], [1, N]])
        dv = sl(Dt, 0, [[N, N], [1, N]])
        nc.vector.tensor_tensor(out=dv, in0=tb, in1=ob, op=ALU.subtract)
        nc.scalar.activation(Dt[:], Dt[:], AF.Square)

        # ---- interleaved forward + reverse wavefront DPs ----
        def step(k, rev, A3t, m1t, mt):
            i0 = max(1, k - N)
            i1 = min(k - 1, N)
            L = i1 - i0 + 1
            if not rev:
                rb = S * i0 + (k - i0)            # = 128*i0 + k
                rs = S - 1                         # +128
                db = (i0 - 1) * N + (k - i0 - 1)   # = 127*i0 + k - 129
                ds = N - 1                         # +127
                Rbuf = Rt
            else:
                rb = (RSZ - 1) - (S * i0 + (k - i0))
                rs = -(S - 1)
                # D'(i',j') = D[N+1-i', N+1-j'] -> 0-idx flat (N-i')*N+(N-j')
                db = (N - i0) * N + (N - (k - i0))
                ds = -(N - 1)
                Rbuf = R2
            # relative offsets of u/l/diag from the output cell
            du = -S if not rev else S
            dl = -1 if not rev else 1
            dd = -(S + 1) if not rev else (S + 1)

            u = sl(Rbuf, rb + du, [[rs, L]])
            l = sl(Rbuf, rb + dl, [[rs, L]])
            dg = sl(Rbuf, rb + dd, [[rs, L]])
            dsl = sl(Dt, db, [[ds, L]])

            m1 = m1t[:, 0:L]
            m = mt[:, 0:L]
            nc.vector.tensor_tensor(out=m1, in0=u, in1=l, op=ALU.min)
            nc.vector.tensor_tensor(out=m, in0=m1, in1=dg, op=ALU.min)

            # A3 interleaved [L,3]: {m-u, m-l} in one broadcast op, {m-dg}
            a = A3t[:]
            A2out = AP(a.tensor, a.offset, [list(a.ap[0]), [3, L], [1, 2]])
            mb = AP(mt[:].tensor, mt[:].offset,
                    [list(mt[:].ap[0]), [1, L], [0, 2]])
            # element (t, w): w=0 -> u(t) at rb+du+rs*t ; w=1 -> l(t) at
            # rb+dl+rs*t = u(t) + (dl - du)
            ul2 = AP(Rbuf[:].tensor, Rbuf[:].offset + rb + du,
                     [list(Rbuf[:].ap[0]), [rs, L], [dl - du, 2]])
            nc.vector.tensor_tensor(out=A2out, in0=mb, in1=ul2,
                                    op=ALU.subtract)
            A1out = AP(a.tensor, a.offset + 2, [list(a.ap[0]), [3, L]])
            nc.gpsimd.tensor_sub(A1out, m, dg)

            nc.scalar.activation(A3t[:, 0:3 * L], A3t[:, 0:3 * L], AF.Exp,
                                 scale=INV_G)

            ss = A3t[:, 384:384 + L]
            A3r = AP(a.tensor, a.offset, [list(a.ap[0]), [3, L], [1, 3]])
            nc.vector.tensor_reduce(ss, A3r, axis=mybir.AxisListType.X,
                                    op=ALU.add)
            nc.scalar.activation(ss, ss, AF.Ln)

            mdc = m1t[:, 0:L]  # m1 is dead; reuse as mdc
            nc.gpsimd.tensor_add(mdc, m, dsl)

            out_sl = sl(Rbuf, rb, [[rs, L]])
            nc.vector.scalar_tensor_tensor(out=out_sl, in0=ss,
                                           scalar=-GAMMA, in1=mdc,
                                           op0=ALU.mult, op1=ALU.add)

        for k in range(2, 2 * N + 1):
            b = k % NB
            step(k, False, A3w[0][b], m1w[0][b], mw[0][b])
            step(k, True, A3w[1][b], m1w[1][b], mw[1][b])

        # ---- E phase ----
        nc.vector.tensor_scalar(out=rnnI[:], in0=Rt[:, RSZ - 1:RSZ],
                                scalar1=INV_G, scalar2=0.0, op0=ALU.mult,
                                op1=ALU.add)
        RC = 32
        for c in range(4):
            i0 = 1 + RC * c
            Dv = sl(Dt, (i0 - 1) * N, [[N, RC], [1, N]])
            Rv = sl(Rt, S * i0 + 1, [[S, RC], [1, N]])
            # Rbar(i,j) lives at R2 flat S*i + j - (S+1)
            R2v = sl(R2, S * i0 + 1 - (S + 1), [[S, RC], [1, N]])
            nc.vector.tensor_tensor(out=Dv, in0=Dv, in1=Rv, op=ALU.subtract)
            nc.vector.tensor_tensor(out=Dv, in0=Dv, in1=R2v,
                                    op=ALU.subtract)
            nc.scalar.activation(Dv, Dv, AF.Exp, scale=INV_G, bias=rnnI[:])
            # omega view: idx = (j - i) + N ; row i0+r, col j=1..N
            SQv = AP(SQf[:].tensor, SQf[:].offset + (1 - i0 + N),
                     [list(SQf[:].ap[0]), [-1, RC], [1, N]])
            nc.vector.scalar_tensor_tensor(out=Dv, in0=Dv, scalar=1.0,
                                           in1=SQv, op0=ALU.mult,
                                           op1=ALU.mult,
                                           accum_out=acc[:, c:c + 1])

        nc.vector.tensor_copy(outt[:, 0:1], Rt[:, RSZ - 1:RSZ])
        nc.vector.tensor_reduce(outt[:, 1:2], acc[:],
                                axis=mybir.AxisListType.X, op=ALU.add)
        nc.sync.dma_start(out_ap[:], outt[:])


_PROGRAM = None


def _get_program():
    global _PROGRAM
    if _PROGRAM is not None:
        return _PROGRAM
    import concourse.bacc as bacc
    import concourse.tile as tile
    import concourse.mybir as mybir

    nc = bacc.Bacc(
        "TRN2",
        target_bir_lowering=False,
        debug=False,
        enable_asserts=False,
        num_devices=NCORES,
    )
    t_ap = nc.dram_tensor("t", [128, N], mybir.dt.float32,
                          kind="ExternalInput").ap()
    o_ap = nc.dram_tensor("o", [128, N], mybir.dt.float32,
                          kind="ExternalInput").ap()
    out_ap = nc.dram_tensor("out", [128, 2], mybir.dt.float32,
                            kind="ExternalOutput").ap()
    with tile.TileContext(nc, trace_sim=False) as tc:
        build_kernel(tc, out_ap, t_ap, o_ap)
    nc.compile()
    _PROGRAM = nc
    return nc


def prep_in_maps(outputs, targets):
    B, Nn, F = outputs.shape  # 128, 128, 8
    assert (B, Nn, F) == (128, 128, 8)
    t = np.ascontiguousarray(
        np.asarray(targets, np.float32).transpose(0, 2, 1).reshape(B * F, Nn))
    o = np.ascontiguousarray(
        np.asarray(outputs, np.float32).transpose(0, 2, 1).reshape(B * F, Nn))

    per = B * F // NCORES  # 128 problems per core
    return [
        {"t": t[c * per:(c + 1) * per], "o": o[c * per:(c + 1) * per]}
        for c in range(NCORES)
    ]


def kernel(outputs, targets):
    from concourse.bass_utils import run_bass_kernel_spmd

    B, Nn, F = outputs.shape
    in_maps = prep_in_maps(outputs, targets)
    nc = _get_program()
    res = run_bass_kernel_spmd(nc, in_maps, core_ids=list(range(NCORES)))
    outs = np.concatenate([r["out"] for r in res.results], axis=0)  # (1024, 2)
    vals = outs[:, 0].astype(np.float64)
    temp = outs[:, 1].astype(np.float64)
    loss_shape = np.float32(vals.mean())
    loss_temporal = np.float32(temp.mean() / (Nn * Nn))
    loss = np.float32(0.5 * loss_shape + 0.5 * loss_temporal)
    return loss, loss_shape, loss_temporal
